# revision 40
# baseline (speedup 1.0000x reference)
"""Trainium2 Bass kernel for nn_Attention_54030688584207.

Single-head attention block:
    h = LN(x^T) ; qkv = h @ W^T + b ; S = q k^T / sqrt(N) + position
    out = softmax(S) @ v, returned as [B, C, N].

Sharding: 8 cores = 4 batches x 2 query-halves, no collectives. Each core
receives its batch's x rotated so its own 1024 query tokens come first and
computes q for its half plus full K/V for the batch (K/V replicated within
the pair), then scores/softmax/PV for its 1024 query rows.

LayerNorm is folded into the QKV epilogues instead of materializing h:
    qkv[d,n] = rstd[n]*( (W'x)[d,n] - mu[n]*wsum[d] ) + b'[d]
so all projection matmuls run on raw (bf16) x with no LN dependency; the
LN statistics (via ones-matmul column sums on the PE) only gate the cheap
DVE epilogues. Softmax skips max-subtraction (scores are O(5), safe in
f32/bf16) so exp(S^T) feeds PV directly as the stationary operand; row
sums come from a ones-column matmul and the division is folded into the
PSUM->SBUF output scale.

Device layouts (per core):
    x_sh  [C=1024, N=2048] bf16  channels x tokens (token-rotated)
    w_t   [C=1024, 3C=3072] bf16 W'^T (gamma/SCALE folded on host)
    bias  [3072] f32             b' (beta folded, q-part scaled)
    pos_t [N=2048, MY=1024] bf16 position^T (rows in local key order)
    out   [MY=1024, C=1024] f32  out[i, c]  (host transposes back)
"""

import os
import sys

for _p in ("/opt/trn_rl_repo",):
    if _p not in sys.path and os.path.isdir(_p):
        sys.path.insert(0, _p)

import numpy as np
import ml_dtypes

import concourse.bass as bass
import concourse.tile as tile
from concourse import bacc, mybir
from concourse.bass import ts, ds
from concourse.bass_utils import run_bass_kernel_spmd

FP = mybir.dt.float32
BF = mybir.dt.bfloat16
AF = mybir.ActivationFunctionType

B = 4
C = 1024
N = 2048
MY = 1024  # query rows per core
D3 = 3 * C
NCH = C // 128   # 8 channel chunks
NJT = N // 128   # 16 key tiles
NIB = MY // 128  # 8 query blocks
NTC = N // 512   # 4 token chunks
LN_EPS = 1e-5
SCALE = 1.0 / np.sqrt(N)


def build_kernel(rep=1, qk_bias=False):
    nc = bacc.Bacc("TRN2", target_bir_lowering=False, debug=False, num_devices=8)
    x_ext = nc.declare_dram_parameter("x_sh", [C, N], BF, isOutput=False)
    wt_ext = nc.declare_dram_parameter("w_t", [C, D3], BF, isOutput=False)
    b_ext = nc.declare_dram_parameter("bias", [D3], FP, isOutput=False)
    ws_ext = nc.declare_dram_parameter("wsum", [D3], FP, isOutput=False)
    pos_ext = nc.declare_dram_parameter("pos_t", [N, MY], BF, isOutput=False)
    out_ext = nc.declare_dram_parameter("out", [MY, C], FP, isOutput=True)

    x_r = x_ext.ap().rearrange("(a p) n -> p a n", p=128)      # [128, 8, N]
    wt_r = wt_ext.ap().rearrange("(a p) d -> p a d", p=128)    # [128, 8, D3]
    b_r = b_ext.ap().rearrange("(a p) -> p a", p=128)          # [128, 24]
    ws_r = ws_ext.ap().rearrange("(a p) -> p a", p=128)        # [128, 24]

    with tile.TileContext(nc) as tc:
      for _r in range(rep):
        with (
            tc.tile_pool(name=f"res{_r}", bufs=1) as res,
            tc.tile_pool(name=f"statb{_r}", bufs=2) as statb,
            tc.tile_pool(name=f"pospool{_r}", bufs=2) as pospool,
            tc.tile_pool(name=f"xsqp{_r}", bufs=3) as xsqp,
            tc.tile_pool(name=f"scr{_r}", bufs=2) as scr,
            tc.tile_pool(name=f"rows{_r}", bufs=1) as rows,
            tc.tile_pool(name=f"small{_r}", bufs=2) as small,
            tc.tile_pool(name=f"dramp{_r}", bufs=1, space="DRAM") as dramp,
            tc.tile_pool(name=f"psum{_r}", bufs=1, space="PSUM") as psum,
        ):
            # ---- resident tiles ----
            xh = res.tile([128, NCH, N], BF, tag="big")       # raw x (bf16)
            qs = res.tile([128, NCH, MY], BF, tag="qs")       # q^T  [c, i]
            ks = res.tile([128, NCH, N], BF, tag="ks")        # k^T  [c, j]
            vs = res.tile([128, NJT, C], BF, tag="vs")        # v    [j, c]
            wqk = res.tile([128, NCH, 2 * C], BF, tag="wqk")  # W'^T q,k cols
            wv = res.tile([128, NCH, C], BF, tag="wv")        # W'^T v cols

            ones_b = rows.tile([128, 1], BF, tag="ones_b")
            nc.vector.memset(ones_b[:], 1.0)

            eps_t = rows.tile([1, 1], FP, tag="eps")
            nc.vector.memset(eps_t[:], LN_EPS)

            # LN stat broadcasts (bf16): -mu*rstd and rstd along tokens
            nmr_b = statb.tile([128, N], BF, tag="statmb", name="nmr_b")
            rstd_b = statb.tile([128, N], BF, tag="statmb", name="rstd_b")
            # per-token-tile columns for the v epilogue (via DRAM bounce)
            mu_col = rows.tile([128, NJT], FP, tag="mu_col")
            nrstd_col = rows.tile([128, NJT], FP, tag="nrstd_col")
            mu_dram = dramp.tile([1, N], FP, tag="mu_dram")
            nrstd_dram = dramp.tile([1, N], FP, tag="nrstd_dram")

            # ---- load x and weights (interleaved so q-weights arrive early) ----
            for c in range(NCH):
                nc.sync.dma_start(xh[:, c, ts(0, 512)], x_r[:, c, ts(0, 512)])
            nc.sync.dma_start(wqk[:, :, ds(0, 512)], wt_r[:, :, ds(0, 512)])
            bias_sb = rows.tile([128, 24], FP, tag="bias")
            nc.sync.dma_start(bias_sb[:], b_r)
            wsum_sb = rows.tile([128, 24], FP, tag="wsum")
            nc.sync.dma_start(wsum_sb[:], ws_r)
            for c in range(NCH):
                nc.sync.dma_start(xh[:, c, ts(1, 512)], x_r[:, c, ts(1, 512)])
            nc.sync.dma_start(wqk[:, :, ds(512, 512)], wt_r[:, :, ds(512, 512)])
            for t in range(2, NTC):
                for c in range(NCH):
                    nc.sync.dma_start(xh[:, c, ts(t, 512)], x_r[:, c, ts(t, 512)])
            for piece in range(2):
                nc.sync.dma_start(wqk[:, :, ds(C + piece * 512, 512)],
                                  wt_r[:, :, ds(C + piece * 512, 512)])
            nc.sync.dma_start(wv[:], wt_r[:, :, ds(2 * C, C)])

            # v-bias + v-weight-colsum broadcast rows [1, C] -> [128, C] (bf16)
            bvrow = statb.tile([1, C], BF, tag="statb", name="bvrow")
            nc.gpsimd.dma_start(bvrow[:], b_ext.ap()[ds(2 * C, C)].rearrange("(o c) -> o c", o=1))
            bv_b = rows.tile([128, C], BF, tag="bvb")
            nc.gpsimd.partition_broadcast(bv_b[:], bvrow[:])
            wvrow = statb.tile([1, C], BF, tag="statb", name="wvrow")
            nc.gpsimd.dma_start(wvrow[:], ws_ext.ap()[ds(2 * C, C)].rearrange("(o c) -> o c", o=1))
            wvsum_b = rows.tile([128, C], BF, tag="wvsb")
            nc.gpsimd.partition_broadcast(wvsum_b[:], wvrow[:])

            # ---- Phase A: LN stats per 512-token chunk ----
            def stats_chunk(t):
                    ps_s = psum.tile([1, 512], FP, tag="w", bufs=8, name=f"ps_s{t}")
                    ps_q = psum.tile([1, 512], FP, tag="w", bufs=8, name=f"ps_q{t}")
                    for c in range(NCH):
                        xsq = xsqp.tile([128, 512], BF, tag="xsq", name=f"xsq{t}_{c}")
                        nc.vector.tensor_mul(xsq[:], xh[:, c, ts(t, 512)],
                                             xh[:, c, ts(t, 512)])
                        nc.tensor.matmul(
                            ps_s[:], ones_b[:], xh[:, c, ts(t, 512)],
                            start=(c == 0), stop=(c == NCH - 1))
                        nc.tensor.matmul(
                            ps_q[:], ones_b[:], xsq[:],
                            start=(c == 0), stop=(c == NCH - 1))
                    # mu = s/C ; var = q/C - mu^2 ; rstd = exp(-0.5 ln(var+eps))
                    mu_c = small.tile([1, 512], FP, tag="mu_c", name=f"mu_c{t}")
                    nc.scalar.mul(mu_c[:], ps_s[:], 1.0 / C)
                    tmp = small.tile([1, 512], FP, tag="tmp", name=f"tmp{t}")
                    nc.vector.tensor_mul(tmp[:], mu_c[:], mu_c[:])
                    nc.vector.scalar_tensor_tensor(
                        tmp[:], ps_q[:], 1.0 / C, tmp[:],
                        op0=mybir.AluOpType.mult, op1=mybir.AluOpType.subtract)
                    nc.scalar.activation(tmp[:], tmp[:], AF.Ln, bias=eps_t[:])
                    rstd_f = small.tile([1, 512], FP, tag="rstd_f", name=f"rstd_f{t}")
                    nc.scalar.activation(rstd_f[:], tmp[:], AF.Exp, scale=-0.5)
                    rstd_cb = small.tile([1, 512], BF, tag="rstd_cb", name=f"rstd_cb{t}")
                    nc.vector.tensor_copy(rstd_cb[:], rstd_f[:])
                    nmr_cb = small.tile([1, 512], BF, tag="nmr_cb", name=f"nmr_cb{t}")
                    nc.vector.scalar_tensor_tensor(
                        nmr_cb[:], mu_c[:], -1.0, rstd_f[:],
                        op0=mybir.AluOpType.mult, op1=mybir.AluOpType.mult)
                    nc.scalar.mul(tmp[:], rstd_f[:], -1.0)  # tmp = -rstd
                    nc.gpsimd.partition_broadcast(nmr_b[:, ts(t, 512)], nmr_cb[:])
                    nc.gpsimd.partition_broadcast(rstd_b[:, ts(t, 512)], rstd_cb[:])
                    # stage mu and -rstd rows to DRAM for columnization
                    nc.sync.dma_start(mu_dram[0:1, ts(t, 512)], mu_c[:])
                    nc.sync.dma_start(nrstd_dram[0:1, ts(t, 512)], tmp[:])

            # ---- Phase B1: q^T and k^T (weights stationary, c-outer groups) ----
            def qk_group(dts, tlist):
                    pss = {}
                    for dt in dts:
                        for t in tlist:
                            pss[(dt, t)] = psum.tile([128, 512], FP, tag="w",
                                                     bufs=8, name=f"qkv_{dt}_{t}")
                    for c in range(NCH):
                        for dt in dts:
                            for t in tlist:
                                nc.tensor.matmul(
                                    pss[(dt, t)][:], wqk[:, c, ts(dt, 128)],
                                    xh[:, c, ts(t, 512)],
                                    start=(c == 0), stop=(c == NCH - 1))
                    for dt in dts:
                        for t in tlist:
                            # t1 = G + (-mu*rstd)*wsum[d] ; qk = t1 * rstd
                            t1 = scr.tile([128, 512], FP, tag="t1",
                                          name=f"t1_{dt}_{t}")
                            nc.vector.scalar_tensor_tensor(
                                t1[:], nmr_b[:, ts(t, 512)], wsum_sb[:, dt:dt + 1],
                                pss[(dt, t)][:],
                                op0=mybir.AluOpType.mult, op1=mybir.AluOpType.add)
                            dst = (qs[:, dt, ts(t, 512)] if dt < 8
                                   else ks[:, dt - 8, ts(t, 512)])
                            nc.vector.tensor_mul(dst, t1[:], rstd_b[:, ts(t, 512)])
                            if qk_bias:
                                nc.vector.tensor_scalar_add(
                                    dst, dst, bias_sb[:, dt:dt + 1])

            stats_chunk(0)
            for g in range(0, 8, 4):
                qk_group(range(g, g + 4), [0])
            stats_chunk(1)
            for g in range(0, 8, 4):
                qk_group(range(g, g + 4), [1])
            stats_chunk(2)
            stats_chunk(3)
            # columnize: [N] rows -> [128, NJT] (token-tile columns)
            nc.sync.dma_start(
                mu_col[:], mu_dram[:].rearrange("o (f p) -> (o p) f", p=128))
            nc.sync.dma_start(
                nrstd_col[:],
                nrstd_dram[:].rearrange("o (f p) -> (o p) f", p=128))
            for g in range(8, 16):
                qk_group(range(g, g + 1), list(range(NTC)))

            # ---- Phase B2: v (activations stationary) ----
            if True:
                for jt in range(NJT):
                    for cc in range(C // 512):
                        psv = psum.tile([128, 512], FP, tag="w",
                                        bufs=8, name=f"psv_{jt}_{cc}")
                        for c in range(NCH):
                            nc.tensor.matmul(
                                psv[:], xh[:, c, ts(jt, 128)],
                                wv[:, c, ts(cc, 512)],
                                start=(c == 0), stop=(c == NCH - 1))
                        # t1 = wvsum*mu[n] - Gv ; v = t1*(-rstd[n]) + bv
                        t1v = scr.tile([128, 512], FP, tag="t1",
                                       name=f"t1v_{jt}_{cc}")
                        nc.vector.scalar_tensor_tensor(
                            t1v[:], wvsum_b[:, ts(cc, 512)], mu_col[:, jt:jt + 1],
                            psv[:],
                            op0=mybir.AluOpType.mult, op1=mybir.AluOpType.subtract)
                        nc.vector.scalar_tensor_tensor(
                            vs[:, jt, ts(cc, 512)], t1v[:], nrstd_col[:, jt:jt + 1],
                            bv_b[:, ts(cc, 512)],
                            op0=mybir.AluOpType.mult, op1=mybir.AluOpType.add)

            # ---- Phase C: S^T = k^T.T q^T + pos ; exp -> es (bf16) ----
            es = res.tile([128, NJT, MY], BF, tag="big")  # reuses xh slot
            if True:
                for j in range(NJT):
                    pos_tile = pospool.tile([128, MY], BF, tag="pos")
                    nc.sync.dma_start(pos_tile[:], pos_ext[ts(j, 128), :])
                    psS = [psum.tile([128, 512], FP, tag="w", bufs=8,
                                     name=f"psS{j}_{ih}") for ih in range(2)]
                    for c in range(NCH):
                        for ih in range(MY // 512):
                            nc.tensor.matmul(
                                psS[ih][:], ks[:, c, ts(j, 128)],
                                qs[:, c, ts(ih, 512)],
                                start=(c == 0), stop=(c == NCH - 1))
                    for ih in range(2):
                        nc.vector.tensor_add(psS[ih][:], psS[ih][:],
                                             pos_tile[:, ts(ih, 512)])
                        nc.scalar.activation(es[:, j, ts(ih, 512)], psS[ih][:],
                                             AF.Exp)

            # ---- Phase D: out[i, c] = (P^T)^T v / rowsum ----
            if True:
                for i in range(NIB):
                    pso = [psum.tile([128, 512], FP, tag="w", bufs=8,
                                     name=f"pso{i}_{cc}") for cc in range(2)]
                    ps_sum = psum.tile([128, 1], FP, tag="w", bufs=8, name=f"ps_sum{i}")
                    for j in range(NJT):
                        lhsT = es[:, j, ts(i, 128)]
                        for cc in range(C // 512):
                            nc.tensor.matmul(
                                pso[cc][:], lhsT, vs[:, j, ts(cc, 512)],
                                start=(j == 0), stop=(j == NJT - 1))
                        nc.tensor.matmul(
                            ps_sum[:], lhsT, ones_b[:],
                            start=(j == 0), stop=(j == NJT - 1))
                    recip = small.tile([128, 1], FP, tag="recip", name=f"recip{i}")
                    nc.vector.reciprocal(recip[:], ps_sum[:])
                    out_t = statb.tile([128, C], FP, tag="statb", name=f"out_t{i}")
                    for cc in range(2):
                        nc.vector.tensor_scalar_mul(out_t[:, ts(cc, 512)],
                                                    pso[cc][:], recip[:])
                    nc.sync.dma_start(out_ext[ts(i, 128), :], out_t[:])

    nc.compile()
    return nc


_NC_CACHE = {}


def _get_nc(qk_bias):
    if qk_bias not in _NC_CACHE:
        _NC_CACHE[qk_bias] = build_kernel(qk_bias=qk_bias)
    return _NC_CACHE[qk_bias]


def prep_in_maps(x, position, ln_gamma, ln_beta, W_qkv, b_qkv):
    """Host-side sharding / layout prep. Returns in_maps for 8 cores."""
    x = np.asarray(x, dtype=np.float32)
    position = np.asarray(position, dtype=np.float32)
    ln_gamma = np.asarray(ln_gamma, dtype=np.float32)
    ln_beta = np.asarray(ln_beta, dtype=np.float32)
    W_qkv = np.asarray(W_qkv, dtype=np.float32)
    b_qkv = np.asarray(b_qkv, dtype=np.float32)

    # Fold gamma into W columns, beta into bias; fold SCALE into q slice.
    Wp = W_qkv * ln_gamma[None, :]
    bp = b_qkv + W_qkv @ ln_beta
    Wp[:C] *= SCALE
    bp[:C] *= SCALE
    w_t = np.ascontiguousarray(Wp.T).astype(ml_dtypes.bfloat16)  # [C, 3C]
    wsum = np.ascontiguousarray(Wp.astype(ml_dtypes.bfloat16).astype(np.float32).sum(axis=1),
                                dtype=np.float32)

    in_maps = []
    for core in range(8):
        b, s = divmod(core, 2)
        if s == 0:
            x_sh = x[b]
            pos_rot = position
        else:
            x_sh = np.roll(x[b], -MY, axis=1)
            pos_rot = np.roll(position, -MY, axis=1)
        pos_t = np.ascontiguousarray(pos_rot[s * MY:(s + 1) * MY, :].T)  # [N, MY]
        in_maps.append({
            "x_sh": np.ascontiguousarray(x_sh).astype(ml_dtypes.bfloat16),
            "w_t": w_t,
            "bias": bp,
            "wsum": wsum,
            "pos_t": pos_t.astype(ml_dtypes.bfloat16),
        })
    return in_maps


def kernel(x, position, ln_gamma, ln_beta, W_qkv, b_qkv):
    in_maps = prep_in_maps(x, position, ln_gamma, ln_beta, W_qkv, b_qkv)
    nc = _get_nc(bool(np.abs(in_maps[0]["bias"][:2 * C]).max() > 0))
    res = run_bass_kernel_spmd(nc, in_maps, core_ids=list(range(8)))
    out = np.empty((B, C, N), dtype=np.float32)
    for core in range(8):
        b, s = divmod(core, 2)
        out[b, :, s * MY:(s + 1) * MY] = res.results[core]["out"].T
    return out


# revision 48
# speedup vs baseline: 1.0015x; 1.0015x over previous
"""Trainium2 Bass kernel for nn_Attention_54030688584207.

Single-head attention block:
    h = LN(x^T) ; qkv = h @ W^T + b ; S = q k^T / sqrt(N) + position
    out = softmax(S) @ v, returned as [B, C, N].

Sharding: 8 cores = 4 batches x 2 query-halves, no collectives. Each core
receives its batch's x rotated so its own 1024 query tokens come first and
computes q for its half plus full K/V for the batch (K/V replicated within
the pair), then scores/softmax/PV for its 1024 query rows.

LayerNorm is folded into the QKV epilogues instead of materializing h:
    qkv[d,n] = rstd[n]*( (W'x)[d,n] - mu[n]*wsum[d] ) + b'[d]
so all projection matmuls run on raw (bf16) x with no LN dependency; the
LN statistics (via ones-matmul column sums on the PE) only gate the cheap
DVE epilogues. Softmax skips max-subtraction (scores are O(5), safe in
f32/bf16) so exp(S^T) feeds PV directly as the stationary operand; row
sums come from a ones-column matmul and the division is folded into the
PSUM->SBUF output scale.

Device layouts (per core):
    x_sh  [C=1024, N=2048] bf16  channels x tokens (token-rotated)
    w_t   [C=1024, 3C=3072] bf16 W'^T (gamma/SCALE folded on host)
    bias  [3072] f32             b' (beta folded, q-part scaled)
    pos_t [N=2048, MY=1024] bf16 position^T (rows in local key order)
    out   [MY=1024, C=1024] f32  out[i, c]  (host transposes back)
"""

import os
import sys

for _p in ("/opt/trn_rl_repo",):
    if _p not in sys.path and os.path.isdir(_p):
        sys.path.insert(0, _p)

import numpy as np
import ml_dtypes

import concourse.bass as bass
import concourse.tile as tile
from concourse import bacc, mybir
from concourse.bass import ts, ds
from concourse.bass_utils import run_bass_kernel_spmd

FP = mybir.dt.float32
BF = mybir.dt.bfloat16
AF = mybir.ActivationFunctionType

B = 4
C = 1024
N = 2048
MY = 1024  # query rows per core
D3 = 3 * C
NCH = C // 128   # 8 channel chunks
NJT = N // 128   # 16 key tiles
NIB = MY // 128  # 8 query blocks
NTC = N // 512   # 4 token chunks
LN_EPS = 1e-5
SCALE = 1.0 / np.sqrt(N)


def build_kernel(rep=1, qk_bias=False):
    nc = bacc.Bacc("TRN2", target_bir_lowering=False, debug=False, num_devices=8)
    x_ext = nc.declare_dram_parameter("x_sh", [C, N], BF, isOutput=False)
    wt_ext = nc.declare_dram_parameter("w_t", [C, D3], BF, isOutput=False)
    b_ext = nc.declare_dram_parameter("bias", [D3], FP, isOutput=False)
    ws_ext = nc.declare_dram_parameter("wsum", [D3], FP, isOutput=False)
    pos_ext = nc.declare_dram_parameter("pos_t", [N, MY], BF, isOutput=False)
    out_ext = nc.declare_dram_parameter("out", [MY, C], FP, isOutput=True)

    x_r = x_ext.ap().rearrange("(a p) n -> p a n", p=128)      # [128, 8, N]
    wt_r = wt_ext.ap().rearrange("(a p) d -> p a d", p=128)    # [128, 8, D3]
    b_r = b_ext.ap().rearrange("(a p) -> p a", p=128)          # [128, 24]
    ws_r = ws_ext.ap().rearrange("(a p) -> p a", p=128)        # [128, 24]

    with tile.TileContext(nc) as tc:
      for _r in range(rep):
        with (
            tc.tile_pool(name=f"res{_r}", bufs=1) as res,
            tc.tile_pool(name=f"statb{_r}", bufs=2) as statb,
            tc.tile_pool(name=f"pospool{_r}", bufs=2) as pospool,
            tc.tile_pool(name=f"xsqp{_r}", bufs=3) as xsqp,
            tc.tile_pool(name=f"scr{_r}", bufs=3) as scr,
            tc.tile_pool(name=f"rows{_r}", bufs=1) as rows,
            tc.tile_pool(name=f"small{_r}", bufs=2) as small,
            tc.tile_pool(name=f"dramp{_r}", bufs=1, space="DRAM") as dramp,
            tc.tile_pool(name=f"psum{_r}", bufs=1, space="PSUM") as psum,
        ):
            # ---- resident tiles ----
            xh = res.tile([128, NCH, N], BF, tag="big")       # raw x (bf16)
            qs = res.tile([128, NCH, MY], BF, tag="qs")       # q^T  [c, i]
            ks = res.tile([128, NCH, N], BF, tag="ks")        # k^T  [c, j]
            vs = res.tile([128, NJT, C], BF, tag="vs")        # v    [j, c]
            wqk = res.tile([128, NCH, 2 * C], BF, tag="wqk")  # W'^T q,k cols
            wv = res.tile([128, NCH, C], BF, tag="wv")        # W'^T v cols

            ones_b = rows.tile([128, 1], BF, tag="ones_b")
            nc.vector.memset(ones_b[:], 1.0)

            eps_t = rows.tile([1, 1], FP, tag="eps")
            nc.vector.memset(eps_t[:], LN_EPS)

            # LN stat broadcasts (bf16): -mu*rstd and rstd along tokens
            nmr_b = statb.tile([128, N], BF, tag="statmb", name="nmr_b")
            rstd_b = statb.tile([128, N], BF, tag="statmb", name="rstd_b")
            # per-token-tile columns for the v epilogue (via DRAM bounce)
            mu_col = rows.tile([128, NJT], FP, tag="mu_col")
            nrstd_col = rows.tile([128, NJT], FP, tag="nrstd_col")
            mu_dram = dramp.tile([1, N], FP, tag="mu_dram")
            nrstd_dram = dramp.tile([1, N], FP, tag="nrstd_dram")

            # ---- load x and weights (interleaved so q-weights arrive early) ----
            for c in range(NCH):
                nc.sync.dma_start(xh[:, c, ts(0, 512)], x_r[:, c, ts(0, 512)])
            nc.sync.dma_start(wqk[:, :, ds(0, 512)], wt_r[:, :, ds(0, 512)])
            bias_sb = rows.tile([128, 24], FP, tag="bias")
            nc.sync.dma_start(bias_sb[:], b_r)
            wsum_sb = rows.tile([128, 24], FP, tag="wsum")
            nc.sync.dma_start(wsum_sb[:], ws_r)
            for c in range(NCH):
                nc.sync.dma_start(xh[:, c, ts(1, 512)], x_r[:, c, ts(1, 512)])
            nc.sync.dma_start(wqk[:, :, ds(512, 512)], wt_r[:, :, ds(512, 512)])
            for t in range(2, NTC):
                for c in range(NCH):
                    nc.sync.dma_start(xh[:, c, ts(t, 512)], x_r[:, c, ts(t, 512)])
            for piece in range(2):
                nc.sync.dma_start(wqk[:, :, ds(C + piece * 512, 512)],
                                  wt_r[:, :, ds(C + piece * 512, 512)])
            nc.sync.dma_start(wv[:], wt_r[:, :, ds(2 * C, C)])

            # v-bias + v-weight-colsum broadcast rows [1, C] -> [128, C] (bf16)
            bvrow = statb.tile([1, C], BF, tag="statb", name="bvrow")
            nc.gpsimd.dma_start(bvrow[:], b_ext.ap()[ds(2 * C, C)].rearrange("(o c) -> o c", o=1))
            bv_b = rows.tile([128, C], BF, tag="bvb")
            nc.gpsimd.partition_broadcast(bv_b[:], bvrow[:])
            wvrow = statb.tile([1, C], BF, tag="statb", name="wvrow")
            nc.gpsimd.dma_start(wvrow[:], ws_ext.ap()[ds(2 * C, C)].rearrange("(o c) -> o c", o=1))
            wvsum_b = rows.tile([128, C], BF, tag="wvsb")
            nc.gpsimd.partition_broadcast(wvsum_b[:], wvrow[:])

            # ---- Phase A: LN stats per 512-token chunk ----
            def stats_chunk(t):
                    ps_s = psum.tile([1, 512], FP, tag="w", bufs=8, name=f"ps_s{t}")
                    ps_q = psum.tile([1, 512], FP, tag="w", bufs=8, name=f"ps_q{t}")
                    for c in range(NCH):
                        xsq = xsqp.tile([128, 512], BF, tag="xsq", name=f"xsq{t}_{c}")
                        nc.vector.tensor_mul(xsq[:], xh[:, c, ts(t, 512)],
                                             xh[:, c, ts(t, 512)])
                        nc.tensor.matmul(
                            ps_s[:], ones_b[:], xh[:, c, ts(t, 512)],
                            start=(c == 0), stop=(c == NCH - 1))
                        nc.tensor.matmul(
                            ps_q[:], ones_b[:], xsq[:],
                            start=(c == 0), stop=(c == NCH - 1))
                    # mu = s/C ; var = q/C - mu^2 ; rstd = exp(-0.5 ln(var+eps))
                    mu_c = small.tile([1, 512], FP, tag="mu_c", name=f"mu_c{t}")
                    nc.scalar.mul(mu_c[:], ps_s[:], 1.0 / C)
                    tmp = small.tile([1, 512], FP, tag="tmp", name=f"tmp{t}")
                    nc.vector.tensor_mul(tmp[:], mu_c[:], mu_c[:])
                    nc.vector.scalar_tensor_tensor(
                        tmp[:], ps_q[:], 1.0 / C, tmp[:],
                        op0=mybir.AluOpType.mult, op1=mybir.AluOpType.subtract)
                    nc.scalar.activation(tmp[:], tmp[:], AF.Ln, bias=eps_t[:])
                    rstd_f = small.tile([1, 512], FP, tag="rstd_f", bufs=1, name=f"rstd_f{t}")
                    nc.scalar.activation(rstd_f[:], tmp[:], AF.Exp, scale=-0.5)
                    rstd_cb = small.tile([1, 512], BF, tag="rstd_cb", name=f"rstd_cb{t}")
                    nc.vector.tensor_copy(rstd_cb[:], rstd_f[:])
                    nmr_cb = small.tile([1, 512], BF, tag="nmr_cb", name=f"nmr_cb{t}")
                    nc.vector.scalar_tensor_tensor(
                        nmr_cb[:], mu_c[:], -1.0, rstd_f[:],
                        op0=mybir.AluOpType.mult, op1=mybir.AluOpType.mult)
                    nc.scalar.mul(tmp[:], rstd_f[:], -1.0)  # tmp = -rstd
                    nc.gpsimd.partition_broadcast(nmr_b[:, ts(t, 512)], nmr_cb[:])
                    nc.gpsimd.partition_broadcast(rstd_b[:, ts(t, 512)], rstd_cb[:])
                    # stage mu and -rstd rows to DRAM for columnization
                    nc.sync.dma_start(mu_dram[0:1, ts(t, 512)], mu_c[:])
                    nc.sync.dma_start(nrstd_dram[0:1, ts(t, 512)], tmp[:])

            # ---- Phase B1: q^T and k^T (weights stationary, c-outer groups) ----
            def qk_group(dts, tlist):
                    pss = {}
                    for dt in dts:
                        for t in tlist:
                            pss[(dt, t)] = psum.tile([128, 512], FP, tag="w",
                                                     bufs=8, name=f"qkv_{dt}_{t}")
                    for c in range(NCH):
                        for dt in dts:
                            for t in tlist:
                                nc.tensor.matmul(
                                    pss[(dt, t)][:], wqk[:, c, ts(dt, 128)],
                                    xh[:, c, ts(t, 512)],
                                    start=(c == 0), stop=(c == NCH - 1))
                    for dt in dts:
                        for t in tlist:
                            # t1 = G + (-mu*rstd)*wsum[d] ; qk = t1 * rstd
                            t1 = scr.tile([128, 512], FP, tag="t1",
                                          name=f"t1_{dt}_{t}")
                            nc.vector.scalar_tensor_tensor(
                                t1[:], nmr_b[:, ts(t, 512)], wsum_sb[:, dt:dt + 1],
                                pss[(dt, t)][:],
                                op0=mybir.AluOpType.mult, op1=mybir.AluOpType.add)
                            dst = (qs[:, dt, ts(t, 512)] if dt < 8
                                   else ks[:, dt - 8, ts(t, 512)])
                            nc.vector.tensor_mul(dst, t1[:], rstd_b[:, ts(t, 512)])
                            if qk_bias:
                                nc.vector.tensor_scalar_add(
                                    dst, dst, bias_sb[:, dt:dt + 1])

            stats_chunk(0)
            for g in range(0, 8, 4):
                qk_group(range(g, g + 4), [0])
            stats_chunk(1)
            for g in range(0, 8, 4):
                qk_group(range(g, g + 4), [1])
            stats_chunk(2)
            stats_chunk(3)
            # columnize: [N] rows -> [128, NJT] (token-tile columns)
            nc.sync.dma_start(
                mu_col[:], mu_dram[:].rearrange("o (f p) -> (o p) f", p=128))
            nc.sync.dma_start(
                nrstd_col[:],
                nrstd_dram[:].rearrange("o (f p) -> (o p) f", p=128))

            # ---- Phase B2: v (activations stationary) ----
            def v_group(jts):
                for jt in jts:
                    for cc in range(C // 512):
                        psv = psum.tile([128, 512], FP, tag="w",
                                        bufs=8, name=f"psv_{jt}_{cc}")
                        for c in range(NCH):
                            nc.tensor.matmul(
                                psv[:], xh[:, c, ts(jt, 128)],
                                wv[:, c, ts(cc, 512)],
                                start=(c == 0), stop=(c == NCH - 1))
                        # t1 = wvsum*mu[n] - Gv ; v = t1*(-rstd[n]) + bv
                        t1v = scr.tile([128, 512], FP, tag="t1",
                                       name=f"t1v_{jt}_{cc}")
                        nc.vector.scalar_tensor_tensor(
                            t1v[:], wvsum_b[:, ts(cc, 512)], mu_col[:, jt:jt + 1],
                            psv[:],
                            op0=mybir.AluOpType.mult, op1=mybir.AluOpType.subtract)
                        nc.vector.scalar_tensor_tensor(
                            vs[:, jt, ts(cc, 512)], t1v[:], nrstd_col[:, jt:jt + 1],
                            bv_b[:, ts(cc, 512)],
                            op0=mybir.AluOpType.mult, op1=mybir.AluOpType.add)

            for gi, g in enumerate(range(8, 16)):
                qk_group(range(g, g + 1), list(range(NTC)))
                if g % 2 == 1:
                    v_group(range((g - 9) // 2 * 4, (g - 9) // 2 * 4 + 4))

            # ---- Phase C: S^T = k^T.T q^T + pos ; exp -> es (bf16) ----
            es = res.tile([128, NJT, MY], BF, tag="big")  # reuses xh slot
            if True:
                for j in range(NJT):
                    pos_tile = pospool.tile([128, MY], BF, tag="pos")
                    nc.sync.dma_start(pos_tile[:], pos_ext[ts(j, 128), :])
                    psS = [psum.tile([128, 512], FP, tag="w", bufs=8,
                                     name=f"psS{j}_{ih}") for ih in range(2)]
                    for c in range(NCH):
                        for ih in range(MY // 512):
                            nc.tensor.matmul(
                                psS[ih][:], ks[:, c, ts(j, 128)],
                                qs[:, c, ts(ih, 512)],
                                start=(c == 0), stop=(c == NCH - 1))
                    for ih in range(2):
                        nc.vector.tensor_add(psS[ih][:], psS[ih][:],
                                             pos_tile[:, ts(ih, 512)])
                        nc.scalar.activation(es[:, j, ts(ih, 512)], psS[ih][:],
                                             AF.Exp)

            # ---- Phase D: out[i, c] = (P^T)^T v / rowsum ----
            if True:
                for i in range(NIB):
                    pso = [psum.tile([128, 512], FP, tag="w", bufs=8,
                                     name=f"pso{i}_{cc}") for cc in range(2)]
                    ps_sum = psum.tile([128, 1], FP, tag="w", bufs=8, name=f"ps_sum{i}")
                    for j in range(NJT):
                        lhsT = es[:, j, ts(i, 128)]
                        for cc in range(C // 512):
                            nc.tensor.matmul(
                                pso[cc][:], lhsT, vs[:, j, ts(cc, 512)],
                                start=(j == 0), stop=(j == NJT - 1))
                        nc.tensor.matmul(
                            ps_sum[:], lhsT, ones_b[:],
                            start=(j == 0), stop=(j == NJT - 1))
                    recip = small.tile([128, 1], FP, tag="recip", name=f"recip{i}")
                    nc.vector.reciprocal(recip[:], ps_sum[:])
                    out_t = statb.tile([128, C], FP, tag="statb", name=f"out_t{i}")
                    nc.vector.tensor_scalar_mul(out_t[:, ts(0, 512)],
                                                pso[0][:], recip[:])
                    nc.sync.dma_start(out_ext[ts(i, 128), ts(0, 512)],
                                      out_t[:, ts(0, 512)])
                    nc.scalar.activation(out_t[:, ts(1, 512)], pso[1][:],
                                         AF.Copy, scale=recip[:])
                    nc.sync.dma_start(out_ext[ts(i, 128), ts(1, 512)],
                                      out_t[:, ts(1, 512)])

    nc.compile()
    return nc


_NC_CACHE = {}


def _get_nc(qk_bias):
    if qk_bias not in _NC_CACHE:
        _NC_CACHE[qk_bias] = build_kernel(qk_bias=qk_bias)
    return _NC_CACHE[qk_bias]


def prep_in_maps(x, position, ln_gamma, ln_beta, W_qkv, b_qkv):
    """Host-side sharding / layout prep. Returns in_maps for 8 cores."""
    x = np.asarray(x, dtype=np.float32)
    position = np.asarray(position, dtype=np.float32)
    ln_gamma = np.asarray(ln_gamma, dtype=np.float32)
    ln_beta = np.asarray(ln_beta, dtype=np.float32)
    W_qkv = np.asarray(W_qkv, dtype=np.float32)
    b_qkv = np.asarray(b_qkv, dtype=np.float32)

    # Fold gamma into W columns, beta into bias; fold SCALE into q slice.
    Wp = W_qkv * ln_gamma[None, :]
    bp = b_qkv + W_qkv @ ln_beta
    Wp[:C] *= SCALE
    bp[:C] *= SCALE
    w_t = np.ascontiguousarray(Wp.T).astype(ml_dtypes.bfloat16)  # [C, 3C]
    wsum = np.ascontiguousarray(Wp.astype(ml_dtypes.bfloat16).astype(np.float32).sum(axis=1),
                                dtype=np.float32)

    in_maps = []
    for core in range(8):
        b, s = divmod(core, 2)
        if s == 0:
            x_sh = x[b]
            pos_rot = position
        else:
            x_sh = np.roll(x[b], -MY, axis=1)
            pos_rot = np.roll(position, -MY, axis=1)
        pos_t = np.ascontiguousarray(pos_rot[s * MY:(s + 1) * MY, :].T)  # [N, MY]
        in_maps.append({
            "x_sh": np.ascontiguousarray(x_sh).astype(ml_dtypes.bfloat16),
            "w_t": w_t,
            "bias": bp,
            "wsum": wsum,
            "pos_t": pos_t.astype(ml_dtypes.bfloat16),
        })
    return in_maps


def kernel(x, position, ln_gamma, ln_beta, W_qkv, b_qkv):
    in_maps = prep_in_maps(x, position, ln_gamma, ln_beta, W_qkv, b_qkv)
    nc = _get_nc(bool(np.abs(in_maps[0]["bias"][:2 * C]).max() > 0))
    res = run_bass_kernel_spmd(nc, in_maps, core_ids=list(range(8)))
    out = np.empty((B, C, N), dtype=np.float32)
    for core in range(8):
        b, s = divmod(core, 2)
        out[b, :, s * MY:(s + 1) * MY] = res.results[core]["out"].T
    return out


# revision 52
# speedup vs baseline: 1.0106x; 1.0091x over previous
"""Trainium2 Bass kernel for nn_Attention_54030688584207.

Single-head attention block:
    h = LN(x^T) ; qkv = h @ W^T + b ; S = q k^T / sqrt(N) + position
    out = softmax(S) @ v, returned as [B, C, N].

Sharding: 8 cores = 4 batches x 2 query-halves, no collectives. Each core
receives its batch's x rotated so its own 1024 query tokens come first and
computes q for its half plus full K/V for the batch (K/V replicated within
the pair), then scores/softmax/PV for its 1024 query rows.

LayerNorm is folded into the QKV epilogues instead of materializing h:
    qkv[d,n] = rstd[n]*( (W'x)[d,n] - mu[n]*wsum[d] ) + b'[d]
so all projection matmuls run on raw (bf16) x with no LN dependency; the
LN statistics (via ones-matmul column sums on the PE) only gate the cheap
DVE epilogues. Softmax skips max-subtraction (scores are O(5), safe in
f32/bf16) so exp(S^T) feeds PV directly as the stationary operand; row
sums come from a ones-column matmul and the division is folded into the
PSUM->SBUF output scale.

Device layouts (per core):
    x_sh  [C=1024, N=2048] bf16  channels x tokens (token-rotated)
    w_t   [C=1024, 3C=3072] bf16 W'^T (gamma/SCALE folded on host)
    bias  [3072] f32             b' (beta folded, q-part scaled)
    pos_t [N=2048, MY=1024] bf16 position^T (rows in local key order)
    out   [MY=1024, C=1024] f32  out[i, c]  (host transposes back)
"""

import os
import sys

for _p in ("/opt/trn_rl_repo",):
    if _p not in sys.path and os.path.isdir(_p):
        sys.path.insert(0, _p)

import numpy as np
import ml_dtypes

import concourse.bass as bass
import concourse.tile as tile
from concourse import bacc, mybir
from concourse.bass import ts, ds
from concourse.bass_utils import run_bass_kernel_spmd

FP = mybir.dt.float32
BF = mybir.dt.bfloat16
AF = mybir.ActivationFunctionType

B = 4
C = 1024
N = 2048
MY = 1024  # query rows per core
D3 = 3 * C
NCH = C // 128   # 8 channel chunks
NJT = N // 128   # 16 key tiles
NIB = MY // 128  # 8 query blocks
NTC = N // 512   # 4 token chunks
LN_EPS = 1e-5
SCALE = 1.0 / np.sqrt(N)


def build_kernel(rep=1, qk_bias=False):
    nc = bacc.Bacc("TRN2", target_bir_lowering=False, debug=False, num_devices=8)
    x_ext = nc.declare_dram_parameter("x_sh", [C, N], BF, isOutput=False)
    wt_ext = nc.declare_dram_parameter("w_t", [C, D3], BF, isOutput=False)
    b_ext = nc.declare_dram_parameter("bias", [D3], FP, isOutput=False)
    ws_ext = nc.declare_dram_parameter("wsum", [D3], FP, isOutput=False)
    pos_ext = nc.declare_dram_parameter("pos_t", [N, MY], BF, isOutput=False)
    out_ext = nc.declare_dram_parameter("out", [MY, C], FP, isOutput=True)

    x_r = x_ext.ap().rearrange("(a p) n -> p a n", p=128)      # [128, 8, N]
    wt_r = wt_ext.ap().rearrange("(a p) d -> p a d", p=128)    # [128, 8, D3]
    b_r = b_ext.ap().rearrange("(a p) -> p a", p=128)          # [128, 24]
    ws_r = ws_ext.ap().rearrange("(a p) -> p a", p=128)        # [128, 24]

    with tile.TileContext(nc) as tc:
      for _r in range(rep):
        with (
            tc.tile_pool(name=f"res{_r}", bufs=1) as res,
            tc.tile_pool(name=f"statb{_r}", bufs=2) as statb,
            tc.tile_pool(name=f"pospool{_r}", bufs=2) as pospool,
            tc.tile_pool(name=f"xsqp{_r}", bufs=3) as xsqp,
            tc.tile_pool(name=f"scr{_r}", bufs=3) as scr,
            tc.tile_pool(name=f"rows{_r}", bufs=1) as rows,
            tc.tile_pool(name=f"small{_r}", bufs=2) as small,
            tc.tile_pool(name=f"dramp{_r}", bufs=1, space="DRAM") as dramp,
            tc.tile_pool(name=f"psum{_r}", bufs=1, space="PSUM") as psum,
        ):
            # ---- resident tiles ----
            xh = res.tile([128, NCH, N], BF, tag="big")       # raw x (bf16)
            qs = res.tile([128, NCH, MY], BF, tag="qs")       # q^T  [c, i]
            ks = res.tile([128, NCH, N], BF, tag="ks")        # k^T  [c, j]
            vs = res.tile([128, NJT, C], BF, tag="vs")        # v    [j, c]
            wqk = res.tile([128, NCH, 2 * C], BF, tag="wqk")  # W'^T q,k cols
            wv = res.tile([128, NCH, C], BF, tag="wv")        # W'^T v cols

            ones_b = rows.tile([128, 1], BF, tag="ones_b")
            nc.vector.memset(ones_b[:], 1.0)

            eps_t = rows.tile([1, 1], FP, tag="eps")
            nc.vector.memset(eps_t[:], LN_EPS)

            # LN stat broadcasts (bf16): -mu*rstd and rstd along tokens
            nmr_b = statb.tile([128, N], BF, tag="statmb", name="nmr_b")
            rstd_b = statb.tile([128, N], BF, tag="statmb", name="rstd_b")
            # per-token-tile columns for the v epilogue (via DRAM bounce)
            mu_col = rows.tile([128, NJT], FP, tag="mu_col")
            nrstd_col = rows.tile([128, NJT], FP, tag="nrstd_col")
            mu_dram = dramp.tile([1, N], FP, tag="mu_dram")
            nrstd_dram = dramp.tile([1, N], FP, tag="nrstd_dram")

            # ---- load x and weights (interleaved so q-weights arrive early) ----
            for ch in range(4):
                nc.sync.dma_start(xh[:, ds(ch * 2, 2), ts(0, 512)],
                                  x_r[:, ds(ch * 2, 2), ts(0, 512)])
            nc.sync.dma_start(wqk[:, :, ds(0, 512)], wt_r[:, :, ds(0, 512)])
            bias_sb = rows.tile([128, 24], FP, tag="bias")
            nc.sync.dma_start(bias_sb[:], b_r)
            wsum_sb = rows.tile([128, 24], FP, tag="wsum")
            nc.sync.dma_start(wsum_sb[:], ws_r)
            nc.sync.dma_start(xh[:, :, ts(1, 512)], x_r[:, :, ts(1, 512)])
            nc.sync.dma_start(wqk[:, :, ds(512, 512)], wt_r[:, :, ds(512, 512)])
            for t in range(2, NTC):
                nc.sync.dma_start(xh[:, :, ts(t, 512)], x_r[:, :, ts(t, 512)])
            for piece in range(2):
                nc.sync.dma_start(wqk[:, :, ds(C + piece * 512, 512)],
                                  wt_r[:, :, ds(C + piece * 512, 512)])
            nc.sync.dma_start(wv[:], wt_r[:, :, ds(2 * C, C)])

            # v-bias + v-weight-colsum broadcast rows [1, C] -> [128, C] (bf16)
            bvrow = statb.tile([1, C], BF, tag="statb", name="bvrow")
            nc.gpsimd.dma_start(bvrow[:], b_ext.ap()[ds(2 * C, C)].rearrange("(o c) -> o c", o=1))
            bv_b = rows.tile([128, C], BF, tag="bvb")
            nc.gpsimd.partition_broadcast(bv_b[:], bvrow[:])
            wvrow = statb.tile([1, C], BF, tag="statb", name="wvrow")
            nc.gpsimd.dma_start(wvrow[:], ws_ext.ap()[ds(2 * C, C)].rearrange("(o c) -> o c", o=1))
            wvsum_b = rows.tile([128, C], BF, tag="wvsb")
            nc.gpsimd.partition_broadcast(wvsum_b[:], wvrow[:])

            # ---- Phase A: LN stats per 512-token chunk ----
            def stats_chunk(t):
                    ps_s = psum.tile([1, 512], FP, tag="w", bufs=8, name=f"ps_s{t}")
                    ps_q = psum.tile([1, 512], FP, tag="w", bufs=8, name=f"ps_q{t}")
                    for c in range(NCH):
                        xsq = xsqp.tile([128, 512], BF, tag="xsq", name=f"xsq{t}_{c}")
                        nc.vector.tensor_mul(xsq[:], xh[:, c, ts(t, 512)],
                                             xh[:, c, ts(t, 512)])
                        nc.tensor.matmul(
                            ps_s[:], ones_b[:], xh[:, c, ts(t, 512)],
                            start=(c == 0), stop=(c == NCH - 1))
                        nc.tensor.matmul(
                            ps_q[:], ones_b[:], xsq[:],
                            start=(c == 0), stop=(c == NCH - 1))
                    # mu = s/C ; var = q/C - mu^2 ; rstd = exp(-0.5 ln(var+eps))
                    mu_c = small.tile([1, 512], FP, tag="mu_c", name=f"mu_c{t}")
                    nc.scalar.mul(mu_c[:], ps_s[:], 1.0 / C)
                    tmp = small.tile([1, 512], FP, tag="tmp", name=f"tmp{t}")
                    nc.vector.tensor_mul(tmp[:], mu_c[:], mu_c[:])
                    nc.vector.scalar_tensor_tensor(
                        tmp[:], ps_q[:], 1.0 / C, tmp[:],
                        op0=mybir.AluOpType.mult, op1=mybir.AluOpType.subtract)
                    nc.scalar.activation(tmp[:], tmp[:], AF.Ln, bias=eps_t[:])
                    rstd_f = small.tile([1, 512], FP, tag="rstd_f", bufs=1, name=f"rstd_f{t}")
                    nc.scalar.activation(rstd_f[:], tmp[:], AF.Exp, scale=-0.5)
                    rstd_cb = small.tile([1, 512], BF, tag="rstd_cb", name=f"rstd_cb{t}")
                    nc.vector.tensor_copy(rstd_cb[:], rstd_f[:])
                    nmr_cb = small.tile([1, 512], BF, tag="nmr_cb", name=f"nmr_cb{t}")
                    nc.vector.scalar_tensor_tensor(
                        nmr_cb[:], mu_c[:], -1.0, rstd_f[:],
                        op0=mybir.AluOpType.mult, op1=mybir.AluOpType.mult)
                    nc.scalar.mul(tmp[:], rstd_f[:], -1.0)  # tmp = -rstd
                    nc.gpsimd.partition_broadcast(nmr_b[:, ts(t, 512)], nmr_cb[:])
                    nc.gpsimd.partition_broadcast(rstd_b[:, ts(t, 512)], rstd_cb[:])
                    # stage mu and -rstd rows to DRAM for columnization
                    nc.sync.dma_start(mu_dram[0:1, ts(t, 512)], mu_c[:])
                    nc.sync.dma_start(nrstd_dram[0:1, ts(t, 512)], tmp[:])

            # ---- Phase B1: q^T and k^T (weights stationary, c-outer groups) ----
            def qk_group(dts, tlist):
                    pss = {}
                    for dt in dts:
                        for t in tlist:
                            pss[(dt, t)] = psum.tile([128, 512], FP, tag="w",
                                                     bufs=8, name=f"qkv_{dt}_{t}")
                    for c in range(NCH):
                        for dt in dts:
                            for t in tlist:
                                nc.tensor.matmul(
                                    pss[(dt, t)][:], wqk[:, c, ts(dt, 128)],
                                    xh[:, c, ts(t, 512)],
                                    start=(c == 0), stop=(c == NCH - 1))
                    for dt in dts:
                        for t in tlist:
                            # t1 = G + (-mu*rstd)*wsum[d] ; qk = t1 * rstd
                            t1 = scr.tile([128, 512], FP, tag="t1",
                                          name=f"t1_{dt}_{t}")
                            nc.vector.scalar_tensor_tensor(
                                t1[:], nmr_b[:, ts(t, 512)], wsum_sb[:, dt:dt + 1],
                                pss[(dt, t)][:],
                                op0=mybir.AluOpType.mult, op1=mybir.AluOpType.add)
                            dst = (qs[:, dt, ts(t, 512)] if dt < 8
                                   else ks[:, dt - 8, ts(t, 512)])
                            nc.vector.tensor_mul(dst, t1[:], rstd_b[:, ts(t, 512)])
                            if qk_bias:
                                nc.vector.tensor_scalar_add(
                                    dst, dst, bias_sb[:, dt:dt + 1])

            stats_chunk(0)
            for g in range(0, 8, 4):
                qk_group(range(g, g + 4), [0])
            stats_chunk(1)
            for g in range(0, 8, 4):
                qk_group(range(g, g + 4), [1])
            stats_chunk(2)
            stats_chunk(3)
            # columnize: [N] rows -> [128, NJT] (token-tile columns)
            nc.sync.dma_start(
                mu_col[:], mu_dram[:].rearrange("o (f p) -> (o p) f", p=128))
            nc.sync.dma_start(
                nrstd_col[:],
                nrstd_dram[:].rearrange("o (f p) -> (o p) f", p=128))

            # ---- Phase B2: v (activations stationary) ----
            def v_group(jts):
                for jt in jts:
                    for cc in range(C // 512):
                        psv = psum.tile([128, 512], FP, tag="w",
                                        bufs=8, name=f"psv_{jt}_{cc}")
                        for c in range(NCH):
                            nc.tensor.matmul(
                                psv[:], xh[:, c, ts(jt, 128)],
                                wv[:, c, ts(cc, 512)],
                                start=(c == 0), stop=(c == NCH - 1))
                        # t1 = wvsum*mu[n] - Gv ; v = t1*(-rstd[n]) + bv
                        t1v = scr.tile([128, 512], FP, tag="t1",
                                       name=f"t1v_{jt}_{cc}")
                        nc.vector.scalar_tensor_tensor(
                            t1v[:], wvsum_b[:, ts(cc, 512)], mu_col[:, jt:jt + 1],
                            psv[:],
                            op0=mybir.AluOpType.mult, op1=mybir.AluOpType.subtract)
                        nc.vector.scalar_tensor_tensor(
                            vs[:, jt, ts(cc, 512)], t1v[:], nrstd_col[:, jt:jt + 1],
                            bv_b[:, ts(cc, 512)],
                            op0=mybir.AluOpType.mult, op1=mybir.AluOpType.add)

            for gi, g in enumerate(range(8, 16)):
                qk_group(range(g, g + 1), list(range(NTC)))
                if g % 2 == 1:
                    v_group(range((g - 9) // 2 * 4, (g - 9) // 2 * 4 + 4))

            # ---- Phase C: S^T = k^T.T q^T + pos ; exp -> es (bf16) ----
            es = res.tile([128, NJT, MY], BF, tag="big")  # reuses xh slot
            if True:
                for j in range(NJT):
                    pos_tile = pospool.tile([128, MY], BF, tag="pos")
                    nc.sync.dma_start(pos_tile[:], pos_ext[ts(j, 128), :])
                    psS = [psum.tile([128, 512], FP, tag="w", bufs=8,
                                     name=f"psS{j}_{ih}") for ih in range(2)]
                    for c in range(NCH):
                        for ih in range(MY // 512):
                            nc.tensor.matmul(
                                psS[ih][:], ks[:, c, ts(j, 128)],
                                qs[:, c, ts(ih, 512)],
                                start=(c == 0), stop=(c == NCH - 1))
                    for ih in range(2):
                        nc.vector.tensor_add(psS[ih][:], psS[ih][:],
                                             pos_tile[:, ts(ih, 512)])
                        nc.scalar.activation(es[:, j, ts(ih, 512)], psS[ih][:],
                                             AF.Exp)

            # ---- Phase D: out[i, c] = (P^T)^T v / rowsum ----
            if True:
                for i in range(NIB):
                    pso = [psum.tile([128, 512], FP, tag="w", bufs=8,
                                     name=f"pso{i}_{cc}") for cc in range(2)]
                    ps_sum = psum.tile([128, 1], FP, tag="w", bufs=8, name=f"ps_sum{i}")
                    for j in range(NJT):
                        lhsT = es[:, j, ts(i, 128)]
                        for cc in range(C // 512):
                            nc.tensor.matmul(
                                pso[cc][:], lhsT, vs[:, j, ts(cc, 512)],
                                start=(j == 0), stop=(j == NJT - 1))
                        nc.tensor.matmul(
                            ps_sum[:], lhsT, ones_b[:],
                            start=(j == 0), stop=(j == NJT - 1))
                    recip = small.tile([128, 1], FP, tag="recip", name=f"recip{i}")
                    nc.vector.reciprocal(recip[:], ps_sum[:])
                    out_t = statb.tile([128, C], FP, tag="statb", name=f"out_t{i}")
                    nc.vector.tensor_scalar_mul(out_t[:, ts(0, 512)],
                                                pso[0][:], recip[:])
                    nc.sync.dma_start(out_ext[ts(i, 128), ts(0, 512)],
                                      out_t[:, ts(0, 512)])
                    nc.scalar.activation(out_t[:, ts(1, 512)], pso[1][:],
                                         AF.Copy, scale=recip[:])
                    nc.sync.dma_start(out_ext[ts(i, 128), ts(1, 512)],
                                      out_t[:, ts(1, 512)])

    nc.compile()
    return nc


_NC_CACHE = {}


def _get_nc(qk_bias):
    if qk_bias not in _NC_CACHE:
        _NC_CACHE[qk_bias] = build_kernel(qk_bias=qk_bias)
    return _NC_CACHE[qk_bias]


def prep_in_maps(x, position, ln_gamma, ln_beta, W_qkv, b_qkv):
    """Host-side sharding / layout prep. Returns in_maps for 8 cores."""
    x = np.asarray(x, dtype=np.float32)
    position = np.asarray(position, dtype=np.float32)
    ln_gamma = np.asarray(ln_gamma, dtype=np.float32)
    ln_beta = np.asarray(ln_beta, dtype=np.float32)
    W_qkv = np.asarray(W_qkv, dtype=np.float32)
    b_qkv = np.asarray(b_qkv, dtype=np.float32)

    # Fold gamma into W columns, beta into bias; fold SCALE into q slice.
    Wp = W_qkv * ln_gamma[None, :]
    bp = b_qkv + W_qkv @ ln_beta
    Wp[:C] *= SCALE
    bp[:C] *= SCALE
    w_t = np.ascontiguousarray(Wp.T).astype(ml_dtypes.bfloat16)  # [C, 3C]
    wsum = np.ascontiguousarray(Wp.astype(ml_dtypes.bfloat16).astype(np.float32).sum(axis=1),
                                dtype=np.float32)

    in_maps = []
    for core in range(8):
        b, s = divmod(core, 2)
        if s == 0:
            x_sh = x[b]
            pos_rot = position
        else:
            x_sh = np.roll(x[b], -MY, axis=1)
            pos_rot = np.roll(position, -MY, axis=1)
        pos_t = np.ascontiguousarray(pos_rot[s * MY:(s + 1) * MY, :].T)  # [N, MY]
        in_maps.append({
            "x_sh": np.ascontiguousarray(x_sh).astype(ml_dtypes.bfloat16),
            "w_t": w_t,
            "bias": bp,
            "wsum": wsum,
            "pos_t": pos_t.astype(ml_dtypes.bfloat16),
        })
    return in_maps


def kernel(x, position, ln_gamma, ln_beta, W_qkv, b_qkv):
    in_maps = prep_in_maps(x, position, ln_gamma, ln_beta, W_qkv, b_qkv)
    nc = _get_nc(bool(np.abs(in_maps[0]["bias"][:2 * C]).max() > 0))
    res = run_bass_kernel_spmd(nc, in_maps, core_ids=list(range(8)))
    out = np.empty((B, C, N), dtype=np.float32)
    for core in range(8):
        b, s = divmod(core, 2)
        out[b, :, s * MY:(s + 1) * MY] = res.results[core]["out"].T
    return out


# revision 53
# speedup vs baseline: 1.0165x; 1.0058x over previous
"""Trainium2 Bass kernel for nn_Attention_54030688584207.

Single-head attention block:
    h = LN(x^T) ; qkv = h @ W^T + b ; S = q k^T / sqrt(N) + position
    out = softmax(S) @ v, returned as [B, C, N].

Sharding: 8 cores = 4 batches x 2 query-halves, no collectives. Each core
receives its batch's x rotated so its own 1024 query tokens come first and
computes q for its half plus full K/V for the batch (K/V replicated within
the pair), then scores/softmax/PV for its 1024 query rows.

LayerNorm is folded into the QKV epilogues instead of materializing h:
    qkv[d,n] = rstd[n]*( (W'x)[d,n] - mu[n]*wsum[d] ) + b'[d]
so all projection matmuls run on raw (bf16) x with no LN dependency; the
LN statistics (via ones-matmul column sums on the PE) only gate the cheap
DVE epilogues. Softmax skips max-subtraction (scores are O(5), safe in
f32/bf16) so exp(S^T) feeds PV directly as the stationary operand; row
sums come from a ones-column matmul and the division is folded into the
PSUM->SBUF output scale.

Device layouts (per core):
    x_sh  [C=1024, N=2048] bf16  channels x tokens (token-rotated)
    w_t   [C=1024, 3C=3072] bf16 W'^T (gamma/SCALE folded on host)
    bias  [3072] f32             b' (beta folded, q-part scaled)
    pos_t [N=2048, MY=1024] bf16 position^T (rows in local key order)
    out   [MY=1024, C=1024] f32  out[i, c]  (host transposes back)
"""

import os
import sys

for _p in ("/opt/trn_rl_repo",):
    if _p not in sys.path and os.path.isdir(_p):
        sys.path.insert(0, _p)

import numpy as np
import ml_dtypes

import concourse.bass as bass
import concourse.tile as tile
from concourse import bacc, mybir
from concourse.bass import ts, ds
from concourse.bass_utils import run_bass_kernel_spmd

FP = mybir.dt.float32
BF = mybir.dt.bfloat16
AF = mybir.ActivationFunctionType

B = 4
C = 1024
N = 2048
MY = 1024  # query rows per core
D3 = 3 * C
NCH = C // 128   # 8 channel chunks
NJT = N // 128   # 16 key tiles
NIB = MY // 128  # 8 query blocks
NTC = N // 512   # 4 token chunks
LN_EPS = 1e-5
SCALE = 1.0 / np.sqrt(N)


def build_kernel(rep=1, qk_bias=False):
    nc = bacc.Bacc("TRN2", target_bir_lowering=False, debug=False, num_devices=8)
    x_ext = nc.declare_dram_parameter("x_sh", [C, N], BF, isOutput=False)
    wt_ext = nc.declare_dram_parameter("w_t", [C, D3], BF, isOutput=False)
    b_ext = nc.declare_dram_parameter("bias", [D3], FP, isOutput=False)
    ws_ext = nc.declare_dram_parameter("wsum", [D3], FP, isOutput=False)
    pos_ext = nc.declare_dram_parameter("pos_t", [N, MY], BF, isOutput=False)
    out_ext = nc.declare_dram_parameter("out", [MY, C], FP, isOutput=True)

    x_r = x_ext.ap().rearrange("(a p) n -> p a n", p=128)      # [128, 8, N]
    wt_r = wt_ext.ap().rearrange("(a p) d -> p a d", p=128)    # [128, 8, D3]
    b_r = b_ext.ap().rearrange("(a p) -> p a", p=128)          # [128, 24]
    ws_r = ws_ext.ap().rearrange("(a p) -> p a", p=128)        # [128, 24]

    with tile.TileContext(nc) as tc:
      for _r in range(rep):
        with (
            tc.tile_pool(name=f"res{_r}", bufs=1) as res,
            tc.tile_pool(name=f"statb{_r}", bufs=2) as statb,
            tc.tile_pool(name=f"pospool{_r}", bufs=2) as pospool,
            tc.tile_pool(name=f"xsqp{_r}", bufs=3) as xsqp,
            tc.tile_pool(name=f"scr{_r}", bufs=3) as scr,
            tc.tile_pool(name=f"rows{_r}", bufs=1) as rows,
            tc.tile_pool(name=f"small{_r}", bufs=2) as small,
            tc.tile_pool(name=f"dramp{_r}", bufs=1, space="DRAM") as dramp,
            tc.tile_pool(name=f"psum{_r}", bufs=1, space="PSUM") as psum,
        ):
            # ---- resident tiles ----
            xh = res.tile([128, NCH, N], BF, tag="big")       # raw x (bf16)
            qs = res.tile([128, NCH, MY], BF, tag="qs")       # q^T  [c, i]
            ks = res.tile([128, NCH, N], BF, tag="ks")        # k^T  [c, j]
            vs = res.tile([128, NJT, C], BF, tag="vs")        # v    [j, c]
            wqk = res.tile([128, NCH, 2 * C], BF, tag="wqk")  # W'^T q,k cols
            wv = res.tile([128, NCH, C], BF, tag="wv")        # W'^T v cols

            ones_b = rows.tile([128, 1], BF, tag="ones_b")
            nc.vector.memset(ones_b[:], 1.0)

            eps_t = rows.tile([1, 1], FP, tag="eps")
            nc.vector.memset(eps_t[:], LN_EPS)

            # LN stat broadcasts (bf16): -mu*rstd and rstd along tokens
            nmr_b = statb.tile([128, N], BF, tag="statmb", name="nmr_b")
            rstd_b = statb.tile([128, N], BF, tag="statmb", name="rstd_b")
            # per-token-tile columns for the v epilogue (via DRAM bounce)
            mu_col = rows.tile([128, NJT], FP, tag="mu_col")
            nrstd_col = rows.tile([128, NJT], FP, tag="nrstd_col")
            mu_dram = dramp.tile([1, N], FP, tag="mu_dram")
            nrstd_dram = dramp.tile([1, N], FP, tag="nrstd_dram")

            # ---- load x and weights (interleaved so q-weights arrive early) ----
            for ch in range(4):
                nc.sync.dma_start(xh[:, ds(ch * 2, 2), ts(0, 512)],
                                  x_r[:, ds(ch * 2, 2), ts(0, 512)])
            nc.sync.dma_start(wqk[:, :, ds(0, 512)], wt_r[:, :, ds(0, 512)])
            bias_sb = rows.tile([128, 24], FP, tag="bias")
            nc.sync.dma_start(bias_sb[:], b_r)
            wsum_sb = rows.tile([128, 24], FP, tag="wsum")
            nc.sync.dma_start(wsum_sb[:], ws_r)
            nc.sync.dma_start(xh[:, :, ts(1, 512)], x_r[:, :, ts(1, 512)])
            nc.sync.dma_start(wqk[:, :, ds(512, 512)], wt_r[:, :, ds(512, 512)])
            for t in range(2, NTC):
                nc.sync.dma_start(xh[:, :, ts(t, 512)], x_r[:, :, ts(t, 512)])
            for piece in range(2):
                nc.sync.dma_start(wqk[:, :, ds(C + piece * 512, 512)],
                                  wt_r[:, :, ds(C + piece * 512, 512)])
            nc.sync.dma_start(wv[:], wt_r[:, :, ds(2 * C, C)])

            # v-bias + v-weight-colsum broadcast rows [1, C] -> [128, C] (bf16)
            bvrow = statb.tile([1, C], BF, tag="statb", name="bvrow")
            nc.gpsimd.dma_start(bvrow[:], b_ext.ap()[ds(2 * C, C)].rearrange("(o c) -> o c", o=1))
            bv_b = rows.tile([128, C], BF, tag="bvb")
            nc.gpsimd.partition_broadcast(bv_b[:], bvrow[:])
            wvrow = statb.tile([1, C], BF, tag="statb", name="wvrow")
            nc.gpsimd.dma_start(wvrow[:], ws_ext.ap()[ds(2 * C, C)].rearrange("(o c) -> o c", o=1))
            wvsum_b = rows.tile([128, C], BF, tag="wvsb")
            nc.gpsimd.partition_broadcast(wvsum_b[:], wvrow[:])

            # ---- Phase A: LN stats per 512-token chunk ----
            def stats_chunk(t):
                    ps_s = psum.tile([1, 512], FP, tag="w", bufs=8, name=f"ps_s{t}")
                    ps_q = psum.tile([1, 512], FP, tag="w", bufs=8, name=f"ps_q{t}")
                    for c in range(NCH):
                        xsq = xsqp.tile([128, 512], BF, tag="xsq", name=f"xsq{t}_{c}")
                        nc.scalar.square(xsq[:], xh[:, c, ts(t, 512)])
                        nc.tensor.matmul(
                            ps_s[:], ones_b[:], xh[:, c, ts(t, 512)],
                            start=(c == 0), stop=(c == NCH - 1))
                        nc.tensor.matmul(
                            ps_q[:], ones_b[:], xsq[:],
                            start=(c == 0), stop=(c == NCH - 1))
                    # mu = s/C ; var = q/C - mu^2 ; rstd = exp(-0.5 ln(var+eps))
                    mu_c = small.tile([1, 512], FP, tag="mu_c", name=f"mu_c{t}")
                    nc.scalar.mul(mu_c[:], ps_s[:], 1.0 / C)
                    tmp = small.tile([1, 512], FP, tag="tmp", name=f"tmp{t}")
                    nc.vector.tensor_mul(tmp[:], mu_c[:], mu_c[:])
                    nc.vector.scalar_tensor_tensor(
                        tmp[:], ps_q[:], 1.0 / C, tmp[:],
                        op0=mybir.AluOpType.mult, op1=mybir.AluOpType.subtract)
                    nc.scalar.activation(tmp[:], tmp[:], AF.Ln, bias=eps_t[:])
                    rstd_f = small.tile([1, 512], FP, tag="rstd_f", bufs=1, name=f"rstd_f{t}")
                    nc.scalar.activation(rstd_f[:], tmp[:], AF.Exp, scale=-0.5)
                    rstd_cb = small.tile([1, 512], BF, tag="rstd_cb", name=f"rstd_cb{t}")
                    nc.vector.tensor_copy(rstd_cb[:], rstd_f[:])
                    nmr_cb = small.tile([1, 512], BF, tag="nmr_cb", name=f"nmr_cb{t}")
                    nc.vector.scalar_tensor_tensor(
                        nmr_cb[:], mu_c[:], -1.0, rstd_f[:],
                        op0=mybir.AluOpType.mult, op1=mybir.AluOpType.mult)
                    nc.scalar.mul(tmp[:], rstd_f[:], -1.0)  # tmp = -rstd
                    nc.gpsimd.partition_broadcast(nmr_b[:, ts(t, 512)], nmr_cb[:])
                    nc.gpsimd.partition_broadcast(rstd_b[:, ts(t, 512)], rstd_cb[:])
                    # stage mu and -rstd rows to DRAM for columnization
                    nc.sync.dma_start(mu_dram[0:1, ts(t, 512)], mu_c[:])
                    nc.sync.dma_start(nrstd_dram[0:1, ts(t, 512)], tmp[:])

            # ---- Phase B1: q^T and k^T (weights stationary, c-outer groups) ----
            def qk_group(dts, tlist):
                    pss = {}
                    for dt in dts:
                        for t in tlist:
                            pss[(dt, t)] = psum.tile([128, 512], FP, tag="w",
                                                     bufs=8, name=f"qkv_{dt}_{t}")
                    for c in range(NCH):
                        for dt in dts:
                            for t in tlist:
                                nc.tensor.matmul(
                                    pss[(dt, t)][:], wqk[:, c, ts(dt, 128)],
                                    xh[:, c, ts(t, 512)],
                                    start=(c == 0), stop=(c == NCH - 1))
                    for dt in dts:
                        for t in tlist:
                            # t1 = G + (-mu*rstd)*wsum[d] ; qk = t1 * rstd
                            t1 = scr.tile([128, 512], FP, tag="t1",
                                          name=f"t1_{dt}_{t}")
                            nc.vector.scalar_tensor_tensor(
                                t1[:], nmr_b[:, ts(t, 512)], wsum_sb[:, dt:dt + 1],
                                pss[(dt, t)][:],
                                op0=mybir.AluOpType.mult, op1=mybir.AluOpType.add)
                            dst = (qs[:, dt, ts(t, 512)] if dt < 8
                                   else ks[:, dt - 8, ts(t, 512)])
                            nc.vector.tensor_mul(dst, t1[:], rstd_b[:, ts(t, 512)])
                            if qk_bias:
                                nc.vector.tensor_scalar_add(
                                    dst, dst, bias_sb[:, dt:dt + 1])

            stats_chunk(0)
            for g in range(0, 8, 4):
                qk_group(range(g, g + 4), [0])
            stats_chunk(1)
            for g in range(0, 8, 4):
                qk_group(range(g, g + 4), [1])
            stats_chunk(2)
            stats_chunk(3)
            # columnize: [N] rows -> [128, NJT] (token-tile columns)
            nc.sync.dma_start(
                mu_col[:], mu_dram[:].rearrange("o (f p) -> (o p) f", p=128))
            nc.sync.dma_start(
                nrstd_col[:],
                nrstd_dram[:].rearrange("o (f p) -> (o p) f", p=128))

            # ---- Phase B2: v (activations stationary) ----
            def v_group(jts):
                for jt in jts:
                    for cc in range(C // 512):
                        psv = psum.tile([128, 512], FP, tag="w",
                                        bufs=8, name=f"psv_{jt}_{cc}")
                        for c in range(NCH):
                            nc.tensor.matmul(
                                psv[:], xh[:, c, ts(jt, 128)],
                                wv[:, c, ts(cc, 512)],
                                start=(c == 0), stop=(c == NCH - 1))
                        # t1 = wvsum*mu[n] - Gv ; v = t1*(-rstd[n]) + bv
                        t1v = scr.tile([128, 512], FP, tag="t1",
                                       name=f"t1v_{jt}_{cc}")
                        nc.vector.scalar_tensor_tensor(
                            t1v[:], wvsum_b[:, ts(cc, 512)], mu_col[:, jt:jt + 1],
                            psv[:],
                            op0=mybir.AluOpType.mult, op1=mybir.AluOpType.subtract)
                        nc.vector.scalar_tensor_tensor(
                            vs[:, jt, ts(cc, 512)], t1v[:], nrstd_col[:, jt:jt + 1],
                            bv_b[:, ts(cc, 512)],
                            op0=mybir.AluOpType.mult, op1=mybir.AluOpType.add)

            for gi, g in enumerate(range(8, 16)):
                qk_group(range(g, g + 1), list(range(NTC)))
                if g % 2 == 1:
                    v_group(range((g - 9) // 2 * 4, (g - 9) // 2 * 4 + 4))

            # ---- Phase C: S^T = k^T.T q^T + pos ; exp -> es (bf16) ----
            es = res.tile([128, NJT, MY], BF, tag="big")  # reuses xh slot
            if True:
                for j in range(NJT):
                    pos_tile = pospool.tile([128, MY], BF, tag="pos")
                    nc.sync.dma_start(pos_tile[:], pos_ext[ts(j, 128), :])
                    psS = [psum.tile([128, 512], FP, tag="w", bufs=8,
                                     name=f"psS{j}_{ih}") for ih in range(2)]
                    for c in range(NCH):
                        for ih in range(MY // 512):
                            nc.tensor.matmul(
                                psS[ih][:], ks[:, c, ts(j, 128)],
                                qs[:, c, ts(ih, 512)],
                                start=(c == 0), stop=(c == NCH - 1))
                    for ih in range(2):
                        nc.vector.tensor_add(psS[ih][:], psS[ih][:],
                                             pos_tile[:, ts(ih, 512)])
                        nc.scalar.activation(es[:, j, ts(ih, 512)], psS[ih][:],
                                             AF.Exp)

            # ---- Phase D: out[i, c] = (P^T)^T v / rowsum ----
            if True:
                for i in range(NIB):
                    pso = [psum.tile([128, 512], FP, tag="w", bufs=8,
                                     name=f"pso{i}_{cc}") for cc in range(2)]
                    ps_sum = psum.tile([128, 1], FP, tag="w", bufs=8, name=f"ps_sum{i}")
                    for j in range(NJT):
                        lhsT = es[:, j, ts(i, 128)]
                        for cc in range(C // 512):
                            nc.tensor.matmul(
                                pso[cc][:], lhsT, vs[:, j, ts(cc, 512)],
                                start=(j == 0), stop=(j == NJT - 1))
                        nc.tensor.matmul(
                            ps_sum[:], lhsT, ones_b[:],
                            start=(j == 0), stop=(j == NJT - 1))
                    recip = small.tile([128, 1], FP, tag="recip", name=f"recip{i}")
                    nc.vector.reciprocal(recip[:], ps_sum[:])
                    out_t = statb.tile([128, C], FP, tag="statb", name=f"out_t{i}")
                    nc.vector.tensor_scalar_mul(out_t[:, ts(0, 512)],
                                                pso[0][:], recip[:])
                    nc.sync.dma_start(out_ext[ts(i, 128), ts(0, 512)],
                                      out_t[:, ts(0, 512)])
                    nc.scalar.activation(out_t[:, ts(1, 512)], pso[1][:],
                                         AF.Copy, scale=recip[:])
                    nc.sync.dma_start(out_ext[ts(i, 128), ts(1, 512)],
                                      out_t[:, ts(1, 512)])

    nc.compile()
    return nc


_NC_CACHE = {}


def _get_nc(qk_bias):
    if qk_bias not in _NC_CACHE:
        _NC_CACHE[qk_bias] = build_kernel(qk_bias=qk_bias)
    return _NC_CACHE[qk_bias]


def prep_in_maps(x, position, ln_gamma, ln_beta, W_qkv, b_qkv):
    """Host-side sharding / layout prep. Returns in_maps for 8 cores."""
    x = np.asarray(x, dtype=np.float32)
    position = np.asarray(position, dtype=np.float32)
    ln_gamma = np.asarray(ln_gamma, dtype=np.float32)
    ln_beta = np.asarray(ln_beta, dtype=np.float32)
    W_qkv = np.asarray(W_qkv, dtype=np.float32)
    b_qkv = np.asarray(b_qkv, dtype=np.float32)

    # Fold gamma into W columns, beta into bias; fold SCALE into q slice.
    Wp = W_qkv * ln_gamma[None, :]
    bp = b_qkv + W_qkv @ ln_beta
    Wp[:C] *= SCALE
    bp[:C] *= SCALE
    w_t = np.ascontiguousarray(Wp.T).astype(ml_dtypes.bfloat16)  # [C, 3C]
    wsum = np.ascontiguousarray(Wp.astype(ml_dtypes.bfloat16).astype(np.float32).sum(axis=1),
                                dtype=np.float32)

    in_maps = []
    for core in range(8):
        b, s = divmod(core, 2)
        if s == 0:
            x_sh = x[b]
            pos_rot = position
        else:
            x_sh = np.roll(x[b], -MY, axis=1)
            pos_rot = np.roll(position, -MY, axis=1)
        pos_t = np.ascontiguousarray(pos_rot[s * MY:(s + 1) * MY, :].T)  # [N, MY]
        in_maps.append({
            "x_sh": np.ascontiguousarray(x_sh).astype(ml_dtypes.bfloat16),
            "w_t": w_t,
            "bias": bp,
            "wsum": wsum,
            "pos_t": pos_t.astype(ml_dtypes.bfloat16),
        })
    return in_maps


def kernel(x, position, ln_gamma, ln_beta, W_qkv, b_qkv):
    in_maps = prep_in_maps(x, position, ln_gamma, ln_beta, W_qkv, b_qkv)
    nc = _get_nc(bool(np.abs(in_maps[0]["bias"][:2 * C]).max() > 0))
    res = run_bass_kernel_spmd(nc, in_maps, core_ids=list(range(8)))
    out = np.empty((B, C, N), dtype=np.float32)
    for core in range(8):
        b, s = divmod(core, 2)
        out[b, :, s * MY:(s + 1) * MY] = res.results[core]["out"].T
    return out
